# revision 1
# baseline (speedup 1.0000x reference)
"""Trainium2 Bass kernel for CellPathwayPoolingAggregator (segment mean).

out[b, p] = (1/segment_sizes[p]) * sum_{k: segment_ids[k]==p} x[b, flat_indices[k]]

Strategy (8 cores, sharded by contiguous pathway ranges):
  - Host: split the 1000 pathways into 8 contiguous ranges (<=128 pathways
    each) balancing per-core unique-gene counts. For each core, dedupe its
    gene rows and pack them into contiguous DRAM slabs in float8_e3m4
    (T k-tiles of 128 gene rows, in 2-tile DMA groups laid out so each DMA
    is a perfect 128-partition x 4KB-per-partition contiguous transfer).
    A per-core count matrix S (e3m4, exact for small counts) carries the
    (gene, pathway) multiplicities.
  - Device (per core): memset-fed warmup matmuls ramp the PE p-state while
    plain sequential dma_starts pull the slabs into SBUF; a PE matmul per
    (k-tile, 512-batch bank) accumulates pathway x batch sums into 4 PSUM
    banks (S tile stationary, gathered rows moving, fp32 accumulate).
    DVE/ACT scale rows by 1/segment_sizes into bf16; two stores on separate
    HWDGE queues write the (128, 2048) output slice; host reassembles.

e3m4 quantization of x gives rel err ~1.3e-2 (< 2e-2 tolerance); counts and
1/size scaling stay exact (counts are small ints; scale applied in f32).
"""

import sys

import numpy as np
import ml_dtypes

_TRN_REPO = "/opt/trn_rl_repo"
if _TRN_REPO not in sys.path:
    sys.path.insert(0, _TRN_REPO)

import concourse.bass as bass  # noqa: F401
import concourse.mybir as mybir
import concourse.tile as tile
from concourse import bacc
from concourse.bass_utils import run_bass_kernel_spmd

B, G, P = 2048, 10000, 1000
NCORES = 8
PC = 128          # max pathways per core (psum partition dim)
NB = B // 512     # matmul N-slices per K-tile (4 banks of 512 f32)
NWARM = 12        # PE warmup matmuls (ramp the tensor engine p-state)
NDR = 5           # trailing 2-tile groups processed as e4m3 DoubleRow pairs

F8 = ml_dtypes.float8_e3m4
F8DR = ml_dtypes.float8_e4m3


def _group_sizes(T):
    """k-tiles per DMA group: uniform 2-tile (4KB/partition) groups.
    (A smaller leading group starves the PE at t=1 and resets its p-state
    ramp — measured worse.)"""
    gs = [2] * (T // 2)
    if T % 2:
        gs.append(1)
    return gs


def _split_ranges(seg_sorted, idx_sorted):
    """Contiguous pathway ranges, <=128 pathways each, minimizing the max
    per-core count of UNIQUE genes (which sets T and hence DMA/PE work)."""
    seg_starts = np.searchsorted(seg_sorted, np.arange(P + 1), side="left")

    def feasible(U):
        bounds = [0]
        for c in range(NCORES):
            lo_p = bounds[-1]
            if lo_p >= P:
                return None
            best = lo_p + 1
            hi_cap = min(P, lo_p + PC)
            lo_e = seg_starts[lo_p]
            for hi_p in range(lo_p + 1, hi_cap + 1):
                nu = len(np.unique(idx_sorted[lo_e : seg_starts[hi_p]]))
                if nu <= U:
                    best = hi_p
                else:
                    break
            bounds.append(best)
        return bounds if bounds[-1] >= P else None

    lo_t, hi_t = 1, (len(idx_sorted) + 127) // 128 + 1
    best_bounds = None
    while lo_t <= hi_t:
        mid = (lo_t + hi_t) // 2
        b = feasible(mid * 128)
        if b is not None:
            best_bounds = b
            hi_t = mid - 1
        else:
            lo_t = mid + 1
    if best_bounds is None:
        best_bounds = list(
            np.minimum(np.arange(NCORES + 1) * ((P + NCORES - 1) // NCORES), P)
        )
    best_bounds[-1] = P
    return best_bounds


def _build_schedule(flat_indices, segment_ids):
    seg = np.asarray(segment_ids, dtype=np.int64)
    idx = np.asarray(flat_indices, dtype=np.int64)
    order = np.argsort(seg, kind="stable")
    seg = seg[order]
    idx = idx[order]

    bounds = _split_ranges(seg, idx)
    cores = []
    for c in range(NCORES):
        lo_p, hi_p = bounds[c], bounds[c + 1]
        lo = np.searchsorted(seg, lo_p, side="left")
        hi = np.searchsorted(seg, hi_p, side="left")
        uidx, inv = np.unique(idx[lo:hi], return_inverse=True)
        cores.append((lo_p, hi_p, uidx, inv, seg[lo:hi] - lo_p))

    T = max(1, max((len(u) + 127) // 128 for _, _, u, _, _ in cores))
    T += T % 2  # even T: uniform 2-tile DMA groups (single-tag tile pool)
    Kpad = T * 128

    s_sbs, uidx_pads = [], []
    for lo_p, hi_p, uidx, inv, cols in cores:
        nu = len(uidx)
        # padded unique-gene list; pad rows point at gene 0 but S is zero there
        uidx_pad = np.concatenate([uidx, np.zeros(Kpad - nu, np.int64)])
        S = np.zeros((Kpad, PC), np.float32)
        np.add.at(S, (inv, cols), 1.0)
        S = S.astype(F8)
        s_sbs.append(
            np.ascontiguousarray(
                S.reshape(T, 128, PC).transpose(1, 0, 2).reshape(128, -1)
            )
        )
        uidx_pads.append(uidx_pad)
    return bounds, uidx_pads, s_sbs, T


def _build_program(T):
    nc = bacc.Bacc(
        "TRN2",
        target_bir_lowering=False,
        debug=False,
        num_devices=NCORES,
        num_swdge_queues=1,
    )
    f8, f32, bf16 = mybir.dt.float8e3, mybir.dt.float32, mybir.dt.bfloat16
    f8dr = mybir.dt.float8e4

    gsz = _group_sizes(T)
    NG = len(gsz)
    ndr = min(NDR, max(0, NG - 2))
    dr0 = NG - ndr  # first DoubleRow group index
    slab_ds = [
        nc.dram_tensor(
            f"slab{g}",
            [128, 2, B] if g >= dr0 else [128, gsz[g] * B],
            f8dr if g >= dr0 else f8,
            kind="ExternalInput",
        )
        for g in range(NG)
    ]
    Tn = int(sum(gsz[:dr0]))  # k-tiles handled by normal e3m4 matmuls
    s_d = nc.dram_tensor("smat", [128, Tn * PC], f8, kind="ExternalInput")
    sdr_d = (
        nc.dram_tensor("smatdr", [128, 2 * ndr, PC], f8dr, kind="ExternalInput")
        if ndr
        else None
    )
    inv_d = nc.dram_tensor("invsz", [128, 1], f32, kind="ExternalInput")
    out_d = nc.dram_tensor("out", [PC, B], bf16, kind="ExternalOutput")

    with tile.TileContext(nc) as tc:
        with (
            tc.tile_pool(name="sb", bufs=1) as pool,
            tc.tile_pool(name="slabp", bufs=NG) as gpool,
            tc.tile_pool(name="psum", bufs=1, space="PSUM") as ppool,
        ):
            # Warmup source: memset on the (otherwise idle) Vector engine —
            # no DMA dependency, so the tensor engine starts ramping its
            # p-state immediately.
            wsrc = pool.tile([128, 512], f8, tag="wsrc")
            nc.vector.memset(wsrc[:], 0)

            # smat/invsz on the Scalar HWDGE queue, in parallel with the
            # slab groups on Sync. (GpSimd DMA is SWDGE — too slow here.)
            s_sb = pool.tile([128, Tn * PC], f8, tag="smat")
            nc.scalar.dma_start(s_sb[:], s_d.ap())
            if ndr:
                sdr_sb = pool.tile([128, 2 * ndr, PC], f8dr, tag="smatdr")
                nc.scalar.dma_start(sdr_sb[:], sdr_d.ap())
            inv_sb = pool.tile([128, 1], f32, tag="invsz")
            nc.scalar.dma_start(inv_sb[:], inv_d.ap())

            psb = [
                ppool.tile([128, 512], f32, tag=f"ps{n}", name=f"ps{n}")
                for n in range(NB)
            ]
            wps = ppool.tile([128, 512], f32, tag="pswarm", name="pswarm")

            gts = []
            for g in range(NG):
                if g >= dr0:
                    gt = gpool.tile([128, 2, B], f8dr, tag="gtdr")
                else:
                    gt = gpool.tile([128, gsz[g] * B], f8, tag="gt")
                nc.sync.dma_start(gt[:], slab_ds[g].ap())
                gts.append(gt)

            # Warmup matmuls: ramp the PE p-state while the first slab
            # group + smat are still in flight.
            for _ in range(NWARM):
                nc.tensor.matmul(
                    wps[:], wsrc[:, :128], wsrc[:], start=True, stop=True
                )

            tstarts = np.cumsum([0] + gsz)
            for g in range(dr0):
                gt = gts[g]
                for cc in range(gsz[g]):
                    t = int(tstarts[g]) + cc
                    for n in range(NB):
                        nc.tensor.matmul(
                            psb[n][:],
                            s_sb[:, t * PC : (t + 1) * PC],
                            gt[:, cc * B + n * 512 : cc * B + (n + 1) * 512],
                            start=(t == 0),
                            stop=(t == T - 1 and not ndr),
                        )
            # Trailing groups as e4m3 DoubleRow pairs: each matmul contracts
            # both k-tiles of the pair at 0.5 cycles/row.
            for j in range(ndr):
                gt = gts[dr0 + j]
                for n in range(NB):
                    nc.tensor.matmul(
                        psb[n][:],
                        sdr_sb[:, 2 * j : 2 * j + 2, :],
                        gt[:, :, n * 512 : (n + 1) * 512],
                        start=False,
                        stop=(j == ndr - 1),
                        perf_mode=mybir.MatmulPerfMode.DoubleRow,
                    )

            # Eviction into two bf16 tiles (DVE even banks, ACT odd banks);
            # each pair tile stored with one Sync DMA as soon as both its
            # banks are evicted. Two stores keep dma_start issue cost (~0.6us
            # each) off the tail; four smaller stores measured worse.
            ots = [
                pool.tile([128, 1024], bf16, tag=f"ot{i}", name=f"ot{i}")
                for i in range(2)
            ]
            for n in range(NB):
                ot = ots[n // 2][:, (n % 2) * 512 : (n % 2 + 1) * 512]
                if n % 2 == 1:
                    nc.scalar.activation(
                        ot,
                        psb[n][:],
                        mybir.ActivationFunctionType.Identity,
                        scale=inv_sb[:],
                    )
                else:
                    nc.vector.tensor_scalar_mul(ot, psb[n][:], inv_sb[:])
                if n % 2 == 1:
                    nc.sync.dma_start(
                        out_d.ap()[:, (n // 2) * 1024 : (n // 2 + 1) * 1024],
                        ots[n // 2][:],
                    )
    return nc


def _prepare(gene_set_features, flat_indices, segment_ids, segment_sizes):
    bounds, uidx_pads, s_sbs, T = _build_schedule(flat_indices, segment_ids)
    nc = _build_program(T)
    nc.compile()

    x = np.asarray(gene_set_features, dtype=np.float32)
    xtf = np.ascontiguousarray(x.T)             # (G, B) f32
    xt8 = xtf.astype(F8)                        # e3m4 for normal tiles
    sizes = np.asarray(segment_sizes, dtype=np.float32)
    gsz = _group_sizes(T)
    NG = len(gsz)
    ndr = min(NDR, max(0, NG - 2))
    dr0 = NG - ndr
    Tn = int(sum(gsz[:dr0]))
    tstarts = np.cumsum([0] + gsz)

    in_maps = []
    for c in range(NCORES):
        lo_p, hi_p = bounds[c], bounds[c + 1]
        inv = np.ones((128, 1), np.float32)
        inv[: hi_p - lo_p, 0] = 1.0 / sizes[lo_p:hi_p]
        m = {"invsz": inv, "smat": np.ascontiguousarray(s_sbs[c][:, : Tn * PC])}
        if ndr:
            # counts are small ints — e3m4 -> e4m3 recast is exact
            m["smatdr"] = np.ascontiguousarray(
                s_sbs[c][:, Tn * PC :].astype(F8DR).reshape(128, 2 * ndr, PC)
            )
        up = uidx_pads[c].reshape(T, 128)  # [t, p]
        for g, gs in enumerate(gsz):
            # slab row p holds the gs gene rows for partition p of group g,
            # concatenated: [ktile tstarts[g]+cc, partition p] for cc in gs.
            tiles = up[tstarts[g] : tstarts[g] + gs]        # [gs, 128]
            perm = tiles.T.reshape(-1)                       # [p, cc]
            if g >= dr0:
                m[f"slab{g}"] = np.ascontiguousarray(
                    xtf[perm].astype(F8DR).reshape(128, 2, B)
                )
            else:
                m[f"slab{g}"] = np.ascontiguousarray(
                    xt8[perm].reshape(128, gs * B)
                )
        in_maps.append(m)
    return nc, in_maps, bounds


def kernel(gene_set_features, flat_indices, segment_ids, segment_sizes, _res_hook=None):
    nc, in_maps, bounds = _prepare(
        gene_set_features, flat_indices, segment_ids, segment_sizes
    )
    res = run_bass_kernel_spmd(nc, in_maps, list(range(NCORES)))
    if _res_hook is not None:
        _res_hook(res)
    outT = np.empty((P, B), np.float32)
    for c in range(NCORES):
        lo_p, hi_p = bounds[c], bounds[c + 1]
        outT[lo_p:hi_p] = np.asarray(res.results[c]["out"]).astype(np.float32)[
            : hi_p - lo_p
        ]
    return np.ascontiguousarray(outT.T)



# revision 3
# speedup vs baseline: 1.0071x; 1.0071x over previous
"""Trainium2 Bass kernel for CellPathwayPoolingAggregator (segment mean).

out[b, p] = (1/segment_sizes[p]) * sum_{k: segment_ids[k]==p} x[b, flat_indices[k]]

Strategy (8 cores, balanced non-contiguous pathway assignment):
  - Host: assign the 1000 pathways to 8 cores (<=128 each) with a greedy
    balance that minimizes the max per-core unique-gene count U (which sets
    the DMA/PE work). Dedupe each core's gene rows, quantize to fp8
    (e3m4 for most k-tiles, e4m3 for the trailing DoubleRow pairs), and
    pack them into per-(psum-bank, chunk) DRAM slabs: bank n holds batch
    columns [512n, 512n+512), so the device streams bank 0 fully, then
    bank 1, ... Each chunk is a contiguous [128, ~4KB/partition] DMA.
  - Device (per core): memset-fed warmup matmuls ramp the PE p-state while
    slab chunks stream in on two HWDGE queues (Sync + Vector, alternating);
    smat/invsz load on the Scalar queue. For each bank: matmuls accumulate
    pathway x 512-batch sums into that bank's PSUM tile (S stationary,
    gathered rows moving); normal e3m4 tiles first, then e4m3 DoubleRow
    pairs (2 k-tiles per pass). When a bank finishes, DVE and ACT each
    scale half of it by 1/segment_sizes into bf16 and the 128KB slice is
    stored on the Scalar queue -- all overlapped with the next bank's
    stream. The final chunk is a single DoubleRow group so the tail after
    the last slab byte is one matmul + half-evictions + a 128KB store.

e3m4 quantization of x gives rel err ~1.3e-2; the e4m3 DoubleRow fraction
(2*NDR of T k-tiles) raises it to ~1.7e-2 (< 2e-2 tolerance). Counts and
1/size scaling stay exact (counts are small ints; scale applied in f32).
"""

import sys

import numpy as np
import ml_dtypes

_TRN_REPO = "/opt/trn_rl_repo"
if _TRN_REPO not in sys.path:
    sys.path.insert(0, _TRN_REPO)

import concourse.bass as bass  # noqa: F401
import concourse.mybir as mybir
import concourse.tile as tile
from concourse import bacc
from concourse.bass_utils import run_bass_kernel_spmd

B, G, P = 2048, 10000, 1000
NCORES = 8
PC = 128          # max pathways per core (psum partition dim)
NB = B // 512     # psum banks / batch phases
NWARM = 14        # PE warmup matmuls (ramp the tensor engine p-state)
NDR = 3           # trailing k-tile pairs processed as e4m3 DoubleRow pairs

F8 = ml_dtypes.float8_e3m4
F8DR = ml_dtypes.float8_e4m3


def _assign_pathways(flat_indices, segment_ids):
    """Greedy balanced assignment of pathways to cores, minimizing the max
    per-core unique-gene count (which sets T and hence DMA/PE work)."""
    seg = np.asarray(segment_ids, dtype=np.int64)
    idx = np.asarray(flat_indices, dtype=np.int64)
    order = np.argsort(seg, kind="stable")
    seg, idx = seg[order], idx[order]
    starts = np.searchsorted(seg, np.arange(P + 1), side="left")
    psets = [np.unique(idx[starts[p] : starts[p + 1]]) for p in range(P)]
    sizes = np.array([len(s) for s in psets])

    covered = np.zeros((NCORES, G), dtype=bool)
    ucnt = np.zeros(NCORES, dtype=np.int64)
    npth = np.zeros(NCORES, dtype=np.int64)
    asg = [[] for _ in range(NCORES)]
    for p in np.argsort(-sizes, kind="stable"):
        best, bkey = -1, None
        for c in range(NCORES):
            if npth[c] >= PC:
                continue
            add = int(np.count_nonzero(~covered[c][psets[p]]))
            key = (ucnt[c] + add, npth[c])
            if best < 0 or key < bkey:
                best, bkey = c, key
        covered[best][psets[p]] = True
        ucnt[best] = bkey[0]
        npth[best] += 1
        asg[best].append(int(p))
    return asg, [np.flatnonzero(covered[c]) for c in range(NCORES)]


def _tile_geometry(umax):
    """T k-tiles; last NORMAL tile is ragged (nlast rows), 2*NDR full DR
    tiles at the end. Gene-row order: (Tn-1) full normal tiles, ragged
    tile, DR tiles."""
    T = max((umax + 127) // 128, 2 * NDR + 2)
    ndr = NDR
    Tn = T - 2 * ndr
    nlast = umax - (T - 1) * 128
    assert 1 <= nlast <= 128 and Tn >= 2
    return T, Tn, ndr, nlast


def _chunk_plan(Tn, ndr, bank):
    """Per-bank chunk list: ('n', tile_lo, ntiles) over the Tn-1 full
    normal tiles, ('r',) ragged tile, then ('d', group_lo, ngroups).
    Bank 0 leads with small chunks so the PE starts early; the last bank
    ends with a single DR group so the tail is one matmul."""
    nfull = Tn - 1
    widths = []
    if bank == 0:
        for w in (2, 2, 4):
            if sum(widths) + w <= nfull:
                widths.append(w)
    while sum(widths) < nfull:
        widths.append(min(8, nfull - sum(widths)))
    chunks = []
    lo = 0
    for w in widths:
        chunks.append(("n", lo, w))
        lo += w
    chunks.append(("r",))
    if bank == NB - 1 and ndr >= 2:
        chunks.append(("d", 0, ndr - 1))
        chunks.append(("d", ndr - 1, 1))
    else:
        chunks.append(("d", 0, ndr))
    return chunks


def _build_program(T, Tn, ndr, nlast):
    nc = bacc.Bacc(
        "TRN2",
        target_bir_lowering=False,
        debug=False,
        num_devices=NCORES,
        num_swdge_queues=1,
    )
    f8, f32, bf16 = mybir.dt.float8e3, mybir.dt.float32, mybir.dt.bfloat16
    f8dr = mybir.dt.float8e4

    plans = [_chunk_plan(Tn, ndr, n) for n in range(NB)]
    slab_ds = {}
    for n in range(NB):
        for ci, ch in enumerate(plans[n]):
            if ch[0] == "n":
                shape = [128, ch[2], 512]
                dt = f8
            elif ch[0] == "r":
                shape = [nlast, 512]
                dt = f8
            else:
                shape = [128, ch[2], 2, 512]
                dt = f8dr
            slab_ds[(n, ci)] = nc.dram_tensor(f"sl{n}_{ci}", shape, dt, kind="ExternalInput")

    s_d = nc.dram_tensor("smat", [128, Tn * PC], f8, kind="ExternalInput")
    sdr_d = nc.dram_tensor("smatdr", [128, 2 * ndr, PC], f8dr, kind="ExternalInput")
    inv_d = nc.dram_tensor("invsz", [128, 1], f32, kind="ExternalInput")
    out_d = nc.dram_tensor("out", [PC, B], bf16, kind="ExternalOutput")

    with tile.TileContext(nc) as tc:
        with (
            tc.tile_pool(name="sb", bufs=1) as pool,
            tc.tile_pool(name="psum", bufs=1, space="PSUM") as ppool,
        ):
            # Warmup source on the (otherwise idle) GpSimd engine so DVE is
            # free to issue slab DMAs immediately.
            wsrc = pool.tile([128, 512], f8, tag="wsrc")
            nc.gpsimd.memset(wsrc[:], 0)

            # smat/invsz on the Scalar HWDGE queue, parallel to the slab
            # chunks on Sync+Vector.
            s_sb = pool.tile([128, Tn * PC], f8, tag="smat")
            nc.scalar.dma_start(s_sb[:], s_d.ap())
            sdr_sb = pool.tile([128, 2 * ndr, PC], f8dr, tag="smatdr")
            nc.scalar.dma_start(sdr_sb[:], sdr_d.ap())
            inv_sb = pool.tile([128, 1], f32, tag="invsz")
            nc.scalar.dma_start(inv_sb[:], inv_d.ap())

            psb = [
                ppool.tile([128, 512], f32, tag=f"ps{n}", name=f"ps{n}")
                for n in range(NB)
            ]
            wps = ppool.tile([128, 512], f32, tag="pswarm", name="pswarm")

            # Slab chunk loads, all on the Sync HWDGE ring in global
            # (= PE consumption) order; the Scalar ring stays free for the
            # mid-stream output stores.
            gts = {}
            for n in range(NB):
                for ci, ch in enumerate(plans[n]):
                    if ch[0] == "n":
                        gt = pool.tile([128, ch[2], 512], f8, tag=f"g{n}_{ci}")
                    elif ch[0] == "r":
                        gt = pool.tile([nlast, 512], f8, tag=f"g{n}_{ci}")
                    else:
                        gt = pool.tile([128, ch[2], 2, 512], f8dr, tag=f"g{n}_{ci}")
                    nc.sync.dma_start(gt[:], slab_ds[(n, ci)].ap())
                    gts[(n, ci)] = gt

            # Warmup matmuls: ramp the PE p-state while the first chunks
            # and smat are in flight.
            for _ in range(NWARM):
                nc.tensor.matmul(
                    wps[:], wsrc[:, :128], wsrc[:], start=True, stop=True
                )

            ots = [
                pool.tile([128, 512], bf16, tag=f"ot{n}", name=f"ot{n}")
                for n in range(NB)
            ]
            for n in range(NB):
                first = True
                for ci, ch in enumerate(plans[n]):
                    gt = gts[(n, ci)]
                    if ch[0] == "n":
                        for t in range(ch[2]):
                            tt = ch[1] + t
                            nc.tensor.matmul(
                                psb[n][:],
                                s_sb[:, tt * PC : (tt + 1) * PC],
                                gt[:, t, :],
                                start=first,
                                stop=False,
                            )
                            first = False
                    elif ch[0] == "r":
                        tt = Tn - 1
                        nc.tensor.matmul(
                            psb[n][:],
                            s_sb[:nlast, tt * PC : (tt + 1) * PC],
                            gt[:, :],
                            start=first,
                            stop=False,
                        )
                        first = False
                    else:
                        for j in range(ch[2]):
                            jj = ch[1] + j
                            last_dr = jj == ndr - 1
                            nc.tensor.matmul(
                                psb[n][:],
                                sdr_sb[:, 2 * jj : 2 * jj + 2, :],
                                gt[:, j, :, :],
                                start=False,
                                stop=last_dr,
                                perf_mode=mybir.MatmulPerfMode.DoubleRow,
                            )
                # Evict bank n: DVE scales cols [0,256), ACT cols [256,512)
                # into bf16; one 128KB store per bank on the Scalar queue,
                # all overlapped with bank n+1's stream.
                nc.vector.tensor_scalar_mul(
                    ots[n][:, :256], psb[n][:, :256], inv_sb[:]
                )
                nc.scalar.activation(
                    ots[n][:, 256:],
                    psb[n][:, 256:],
                    mybir.ActivationFunctionType.Identity,
                    scale=inv_sb[:],
                )
                nc.scalar.dma_start(
                    out_d.ap()[:, n * 512 : (n + 1) * 512], ots[n][:]
                )
    return nc


def _build_schedule(flat_indices, segment_ids):
    asg, uidxs = _assign_pathways(flat_indices, segment_ids)
    umax = max(len(u) for u in uidxs)
    T, Tn, ndr, nlast = _tile_geometry(umax)

    seg = np.asarray(segment_ids, dtype=np.int64)
    idx = np.asarray(flat_indices, dtype=np.int64)

    s_mats, sdr_mats, rows_list = [], [], []
    for c in range(NCORES):
        uidx = uidxs[c]
        nu = len(uidx)
        # gene rows in core order, padded to umax with -1 (packed as zeros)
        rows = np.full(umax, -1, dtype=np.int64)
        rows[:nu] = uidx
        rows_list.append(rows)
        gene_pos = np.full(G, -1, dtype=np.int64)
        gene_pos[uidx] = np.arange(nu)

        S = np.zeros((umax, PC), dtype=np.float32)
        for slot, p in enumerate(asg[c]):
            mask = seg == p
            np.add.at(S, (gene_pos[idx[mask]], slot), 1.0)
        # row order: (Tn-1) full tiles, ragged tile (nlast), DR tiles
        nrm = (Tn - 1) * 128 + nlast
        Sn = np.zeros((Tn * 128, PC), dtype=np.float32)
        Sn[: (Tn - 1) * 128] = S[: (Tn - 1) * 128]
        Sn[(Tn - 1) * 128 : (Tn - 1) * 128 + nlast] = S[(Tn - 1) * 128 : nrm]
        s_mats.append(
            np.ascontiguousarray(
                Sn.reshape(Tn, 128, PC).transpose(1, 0, 2).reshape(128, Tn * PC)
            ).astype(F8)
        )
        Sd = S[nrm:].reshape(2 * ndr, 128, PC).transpose(1, 0, 2)
        sdr_mats.append(np.ascontiguousarray(Sd).astype(F8DR))
    return asg, rows_list, s_mats, sdr_mats, (T, Tn, ndr, nlast)


def _prepare(gene_set_features, flat_indices, segment_ids, segment_sizes):
    asg, rows_list, s_mats, sdr_mats, geom = _build_schedule(
        flat_indices, segment_ids
    )
    T, Tn, ndr, nlast = geom
    nc = _build_program(T, Tn, ndr, nlast)
    nc.compile()

    x = np.asarray(gene_set_features, dtype=np.float32)
    xtf = np.ascontiguousarray(x.T)             # (G, B) f32
    xt8 = xtf.astype(F8)                        # e3m4 for normal tiles
    xt8dr = xtf.astype(F8DR)                    # e4m3 for DoubleRow tiles
    zrow8 = np.zeros(B, dtype=F8)
    zrow8dr = np.zeros(B, dtype=F8DR)
    sizes = np.asarray(segment_sizes, dtype=np.float32)
    plans = [_chunk_plan(Tn, ndr, n) for n in range(NB)]
    nrm = (Tn - 1) * 128 + nlast

    in_maps = []
    for c in range(NCORES):
        rows = rows_list[c]
        # full-batch row matrices in tile order (pad rows -> zeros)
        rn = rows[:nrm]
        xrows_n = np.where(rn[:, None] >= 0, xt8[np.maximum(rn, 0)], zrow8)
        rd = rows[nrm:]
        xrows_d = np.where(rd[:, None] >= 0, xt8dr[np.maximum(rd, 0)], zrow8dr)

        inv = np.ones((128, 1), np.float32)
        for slot, p in enumerate(asg[c]):
            inv[slot, 0] = 1.0 / sizes[p]
        m = {
            "invsz": inv,
            "smat": s_mats[c],
            "smatdr": sdr_mats[c],
        }
        for n in range(NB):
            bsl = slice(n * 512, (n + 1) * 512)
            for ci, ch in enumerate(plans[n]):
                if ch[0] == "n":
                    lo = ch[1] * 128
                    blk = xrows_n[lo : lo + ch[2] * 128, bsl]
                    m[f"sl{n}_{ci}"] = np.ascontiguousarray(
                        blk.reshape(ch[2], 128, 512).transpose(1, 0, 2)
                    )
                elif ch[0] == "r":
                    blk = xrows_n[(Tn - 1) * 128 : (Tn - 1) * 128 + nlast, bsl]
                    m[f"sl{n}_{ci}"] = np.ascontiguousarray(blk)
                else:
                    lo = ch[1] * 2 * 128
                    blk = xrows_d[lo : lo + ch[2] * 2 * 128, bsl]
                    m[f"sl{n}_{ci}"] = np.ascontiguousarray(
                        blk.reshape(ch[2], 2, 128, 512).transpose(2, 0, 1, 3)
                    )
        in_maps.append(m)
    return nc, in_maps, asg


def _unshard(res, asg):
    outT = np.empty((P, B), np.float32)
    for c in range(NCORES):
        o = np.asarray(res.results[c]["out"]).astype(np.float32)
        outT[np.asarray(asg[c], dtype=np.int64)] = o[: len(asg[c])]
    return np.ascontiguousarray(outT.T)


def kernel(gene_set_features, flat_indices, segment_ids, segment_sizes, _res_hook=None):
    nc, in_maps, asg = _prepare(
        gene_set_features, flat_indices, segment_ids, segment_sizes
    )
    res = run_bass_kernel_spmd(nc, in_maps, list(range(NCORES)))
    if _res_hook is not None:
        _res_hook(res)
    return _unshard(res, asg)
